# revision 8
# baseline (speedup 1.0000x reference)
"""GAT (graph attention) layer on 8 Trainium2 NeuronCores.

Reference computation (N=8192, F_IN=256, F_OUT=64, alpha=0.2):
    Wh     = h @ W                                  [N, 64]
    f_src  = Wh @ a[:64, 0]                         [N]
    f_dst  = Wh @ a[64:, 0]                         [N]
    e      = leaky_relu(f_src[:,None] + f_dst[None,:], 0.2)
    att    = softmax(where(adj > 0, e, -9e15), axis=1)
    out    = elu(att @ Wh)

Sharding: row-shard the N dimension across 8 cores (1024 query rows per
core); every core computes the full Wh / rhs factors (replicated).

Key algebraic / layout transforms:
 1. exp(lrelu(u)) = exp(0.2 f_src_i) * exp(0.2 f_dst_j) * max(exp(0.8 u), 1).
    The exp(0.2 f_src_i) factor cancels in the softmax ratio;
    exp(0.2 f_dst_j) = b2_j is folded into the matmul rhs
    (rhs_aug[j,:] = b2_j * [Wh_j | 1]); the trailing ones-column makes
    the attention matmul also produce the softmax denominator Z_i.
 2. exp(0.8 u_ij) = s8_i * b8_j is RANK-1 (outer product), so the
    elementwise field is built by one dual-op tensor_scalar
    (mult by per-partition b8_j, then max with 1) -- no ACT exp over the
    N^2 field at all.
 3. The whole elementwise phase runs in TRANSPOSED space [j-part, i-free]
    using a host-transposed adjacency (adjT, bf16).  The masked product
    P^T[j,i] then feeds nc.tensor.matmul directly as the stationary
    operand (lhsT) -- zero PE transposes and zero PSUM->SBUF copies of
    the N^2 field (the baseline's dominant cost).
 4. bf16 for the N^2 tensors: DVE runs tensor_scalar at 4x and
    tensor_tensor at 2x; PE runs bf16 matmuls at 1 cyc/row with FWL.
    Errors cancel between softmax numerator and denominator (same P
    factor), measured rel err ~1e-3 << 2e-2 tolerance.
"""

import sys

sys.path.insert(0, "/opt/trn_rl_repo")

import numpy as np
import ml_dtypes

import concourse.bass as bass  # noqa: F401
import concourse.mybir as mybir
import concourse.tile as tile
from concourse import bacc
from concourse.bass_utils import run_bass_kernel_spmd
from concourse.masks import make_identity

N = 8192
F_IN = 256
F_OUT = 64
N_CORES = 8
ROWS = N // N_CORES  # 1024 query rows per core

F32 = mybir.dt.float32
F32R = mybir.dt.float32r
BF16 = mybir.dt.bfloat16
FP16 = mybir.dt.float16
ALPHA = 2.0 ** -12  # global softmax-invariant scale: keeps fp16 in range
LN_ALPHA = float(np.log(ALPHA))
Act = mybir.ActivationFunctionType
Alu = mybir.AluOpType

MCH = N // 128  # 64 chunks over all rows (j)
LCH = ROWS // 128  # 8 local chunks (i)
NA = F_OUT + 2  # rhs_aug free dim: 64 Wh cols + denominator col + pad

_CACHE = {}


def _build_nc(repeat=1):
    nc = bacc.Bacc(
        "TRN2",
        target_bir_lowering=False,
        debug=False,
        enable_asserts=False,
        num_devices=N_CORES,
    )

    hT = nc.dram_tensor("hT", [F_IN, N], F32, kind="ExternalInput")
    hsT = nc.dram_tensor("hsT", [F_IN, ROWS], F32, kind="ExternalInput")
    adjT = nc.dram_tensor("adjT", [N, ROWS], FP16, kind="ExternalInput")
    W = nc.dram_tensor("W", [F_IN, F_OUT], F32, kind="ExternalInput")
    a = nc.dram_tensor("a", [2 * F_OUT, 1], F32, kind="ExternalInput")
    out = nc.dram_tensor("out", [ROWS, F_OUT], F32, kind="ExternalOutput")

    # DRAM bounce buffer: s8 column -> free-axis row for partition-broadcast
    s8d = nc.dram_tensor("s8d", [ROWS], FP16)

    from contextlib import nullcontext

    with tile.TileContext(nc) as tc:
        rep_ctx = tc.For_i(0, repeat, 1) if repeat > 1 else nullcontext()
        with rep_ctx:
            _kernel_body(nc, tc, hT, hsT, adjT, W, a, out, s8d)

    nc.compile()
    return nc


def _kernel_body(nc, tc, hT, hsT, adjT, W, a, out, s8d):
    with (
        tc.tile_pool(name="consts", bufs=1) as consts,
        tc.tile_pool(name="adjp", bufs=3) as adjp,
        tc.tile_pool(name="wk", bufs=3) as wk,
        tc.tile_pool(name="ep", bufs=2) as ep,
        tc.tile_pool(name="psw", bufs=2, space="PSUM") as psw,
        tc.tile_pool(name="psacc", bufs=1, space="PSUM") as psacc,
    ):
        # ---------------- constants ----------------
        idf = consts.tile([128, 128], F32)
        make_identity(nc, idf)

        # Waug = [W | w_src | w_dst] as [128, 2, 66] f32
        Waug = consts.tile([128, 2, F_OUT + 2], F32)
        nc.sync.dma_start(
            out=Waug[:, :, 0:F_OUT],
            in_=W[:, :].rearrange("(c p) f -> p c f", p=128),
        )
        a2 = consts.tile([F_OUT, 2], F32)
        nc.sync.dma_start(out=a2[:, 0:1], in_=a[0:F_OUT, :])
        nc.sync.dma_start(out=a2[:, 1:2], in_=a[F_OUT : 2 * F_OUT, :])

        WT = consts.tile([F_OUT, 2, 128], F32)
        for rc in range(2):
            wtps = psw.tile([F_OUT, 128], F32, tag="pt")
            nc.tensor.transpose(wtps, Waug[:, rc, 0:F_OUT], idf)
            nc.any.tensor_copy(WT[:, rc, :], wtps)
        for rc in range(2):
            wps = psw.tile([128, 2], F32, tag="pt")
            nc.tensor.matmul(wps, lhsT=WT[:, rc, :], rhs=a2, start=True, stop=True)
            nc.any.tensor_copy(Waug[:, rc, F_OUT : F_OUT + 2], wps)

        # ---------------- own-row f_src -> s8row broadcast ----------------
        hsTs = consts.tile([128, 2, ROWS], F32)
        for kc in range(2):
            nc.sync.dma_start(
                out=hsTs[:, kc, :], in_=hsT[kc * 128 : (kc + 1) * 128, :]
            )
        lnal = consts.tile([128, 1], F32)
        nc.vector.memset(lnal, LN_ALPHA)
        s8col = consts.tile([128, LCH], F32)
        for lc in range(LCH):
            fops = psw.tile([128, 2], F32, tag="fs")
            for kc in range(2):
                nc.tensor.matmul(
                    fops,
                    lhsT=hsTs[:, kc, lc * 128 : (lc + 1) * 128],
                    rhs=Waug[:, kc, F_OUT : F_OUT + 2],
                    start=(kc == 0),
                    stop=(kc == 1),
                )
            # s8 = exp(0.8 * f_src)
            nc.scalar.activation(
                s8col[:, lc : lc + 1], fops[:, 0:1], Act.Exp,
                bias=lnal, scale=0.8,
            )
        s8T_ps = psw.tile([LCH, 128], F32, tag="pt")
        nc.tensor.transpose(s8T_ps, s8col, idf)
        s8T = consts.tile([LCH, 128], FP16)
        nc.any.tensor_copy(s8T, s8T_ps)
        nc.gpsimd.dma_start(out=s8d[:].rearrange("(q p) -> q p", p=128), in_=s8T)
        s8row = consts.tile([128, ROWS], FP16)
        s8d_bc = bass.AP(tensor=s8d, offset=0, ap=[[0, 128], [1, ROWS]])
        nc.gpsimd.dma_start(out=s8row, in_=s8d_bc)

        # ---------------- full Wh -> rhs_aug (pipelined with main loop) ----
        hTs = consts.tile([128, 2, N], F32)
        for kc in range(2):
            for piece in range(8):
                nc.sync.dma_start(
                    out=hTs[:, kc, piece * 1024 : (piece + 1) * 1024],
                    in_=hT[
                        kc * 128 : (kc + 1) * 128, piece * 1024 : (piece + 1) * 1024
                    ],
                )
        b2col = consts.tile([128, MCH], F32)
        b8col = consts.tile([128, MCH], F32)
        rhs_aug = consts.tile([128, MCH, NA], FP16)
        nc.vector.memset(rhs_aug[:, :, F_OUT + 1], 0.0)
        for mc in range(MCH):
            whps = psw.tile([128, F_OUT + 2], F32, tag="wh")
            for kc in range(2):
                nc.tensor.matmul(
                    whps,
                    lhsT=hTs[:, kc, mc * 128 : (mc + 1) * 128],
                    rhs=Waug[:, kc, :],
                    start=(kc == 0),
                    stop=(kc == 1),
                )
            # b2 = exp(0.2 f_dst), b8 = exp(0.8 f_dst)
            nc.scalar.activation(
                b2col[:, mc : mc + 1], whps[:, F_OUT + 1 : F_OUT + 2], Act.Exp,
                scale=0.2,
            )
            nc.scalar.activation(
                b8col[:, mc : mc + 1], whps[:, F_OUT + 1 : F_OUT + 2], Act.Exp,
                scale=0.8,
            )
            # rhs_aug[:, mc, 0:64] = b2 * Wh  (scaled PSUM->SBUF copy on ACT)
            nc.scalar.activation(
                rhs_aug[:, mc, 0:F_OUT], whps[:, 0:F_OUT], Act.Copy,
                scale=b2col[:, mc : mc + 1],
            )
            # denominator column
            nc.vector.tensor_copy(
                rhs_aug[:, mc, F_OUT : F_OUT + 1], b2col[:, mc : mc + 1]
            )

        # ---------------- main loop: P^T = adjT * max(s8_i*b8_j, 1) --------
        acc0 = psacc.tile([128, 4, NA], F32, tag="acc0")
        acc1 = psacc.tile([128, 4, NA], F32, tag="acc1")
        accs = [acc0, acc1]
        for jc in range(MCH):
            adjt = adjp.tile([128, ROWS], FP16, tag="adj")
            nc.sync.dma_start(
                out=adjt, in_=adjT[jc * 128 : (jc + 1) * 128, :]
            )
            Xm = wk.tile([128, ROWS], FP16, tag="xm")
            nc.vector.tensor_scalar(
                Xm, s8row, b8col[:, jc : jc + 1], ALPHA, Alu.mult, Alu.max
            )
            P = wk.tile([128, ROWS], FP16, tag="p")
            nc.vector.tensor_tensor(P, Xm, adjt, Alu.mult)
            for ic in range(LCH):
                nc.tensor.matmul(
                    accs[ic // 4][:, ic % 4, :],
                    lhsT=P[:, ic * 128 : (ic + 1) * 128],
                    rhs=rhs_aug[:, jc, :],
                    start=(jc == 0),
                    stop=(jc == MCH - 1),
                )

        # ---------------- epilogue: h' = S/Z ; out = elu(h') ----------------
        sc = ep.tile([128, LCH, F_OUT], F32, tag="sc")
        for ic in range(LCH):
            acc = accs[ic // 4]
            rz = ep.tile([128, 1], F32, tag="rz")
            nc.vector.reciprocal(rz, acc[:, ic % 4, F_OUT : F_OUT + 1])
            nc.vector.tensor_scalar(
                sc[:, ic, :], acc[:, ic % 4, 0:F_OUT], rz, None, Alu.mult
            )
        # elu(x) = exp(min(x,0)) + max(x,0) - 1
        mn = ep.tile([128, LCH, F_OUT], F32, tag="mn")
        nc.vector.tensor_scalar(mn, sc, 0.0, None, Alu.min)
        em = ep.tile([128, LCH, F_OUT], F32, tag="em")
        nc.scalar.activation(em, mn, Act.Exp)
        rp1 = ep.tile([128, LCH, F_OUT], F32, tag="rp1")
        nc.vector.tensor_scalar(rp1, sc, 0.0, -1.0, Alu.max, Alu.add)
        ob = ep.tile([128, LCH, F_OUT], F32, tag="ob")
        nc.vector.tensor_tensor(ob, em, rp1, Alu.add)
        nc.gpsimd.dma_start(
            out=out[:, :].rearrange("(c p) f -> p c f", p=128), in_=ob
        )


def _get_nc(repeat=1):
    key = ("nc", repeat)
    if key not in _CACHE:
        _CACHE[key] = _build_nc(repeat)
    return _CACHE[key]


def kernel(h, adj, W, a, _collect_results=False, _trace=False):
    h = np.ascontiguousarray(h, dtype=np.float32)
    adj = np.ascontiguousarray(adj, dtype=np.int32)
    W = np.ascontiguousarray(W, dtype=np.float32)
    a = np.ascontiguousarray(a, dtype=np.float32)

    hT = np.ascontiguousarray(h.T)
    adj_bf = adj.astype(np.float16)

    nc = _get_nc()
    in_maps = []
    for c in range(N_CORES):
        sl = slice(c * ROWS, (c + 1) * ROWS)
        in_maps.append(
            {
                "hT": hT,
                "hsT": np.ascontiguousarray(hT[:, sl]),
                "adjT": np.ascontiguousarray(adj_bf[sl].T),
                "W": W,
                "a": a,
            }
        )
    res = run_bass_kernel_spmd(nc, in_maps, list(range(N_CORES)), trace=_trace)
    out = np.concatenate([res.results[c]["out"] for c in range(N_CORES)], axis=0)
    out = np.ascontiguousarray(out, dtype=np.float32)
    if _collect_results:
        return out, res
    return out
